# revision 1
# baseline (speedup 1.0000x reference)
"""Soft decision-tree forward (nn_DTree) on 8 trn2 NeuronCores.

Strategy (pure data parallel, per the sharding hint):
  - shard x row-wise 8 ways; replicate the tiny tree params.
  - per core: z = [x|1|1] @ [W | -c_hi | -c_lo]^T via bf16 PE matmuls into
    fp32 PSUM, g = sigmoid(z) on ACT, then a level-by-level value-tree
    blend on DVE:
       V_k = g_k * (V_{k+1,L} - V_{k+1,R}) + V_{k+1,R}
    with nodes pre-permuted (level-major, left-children-first) so every
    level's children are two contiguous halves of the previous level.
  - x reaches the PE transposed via the DMA xbar: x is cast to bf16 into a
    [rows, 64] DRAM bounce (cols 0-31 = features, 32-33 = bias-ones slots),
    whose [rows/2, 128] view is xbar-transpose-legal (cols % 128 == 0).
    The transposed SBUF buffer holds two interleaved row-classes
    (rows 2j+s at partitions 64s+f); the resulting row permutation of the
    [128, tiles] output is undone on the host (128 KiB reorder).
"""

import numpy as np
import ml_dtypes

import concourse.bass as bass
import concourse.bacc as bacc
import concourse.tile as tile
from concourse import mybir
from concourse.bass_utils import run_bass_kernel_spmd

BF16 = ml_dtypes.bfloat16

F = 32
D = 8
NODES = 255
LEAVES = 256
N_FULL = 262144
N_CORES = 8
ROWS = N_FULL // N_CORES  # 32768 rows per core

# level-major offsets of each level's gates inside the 255-column block
LEVEL_OFF = {7: 0, 6: 128, 5: 192, 4: 224, 3: 240, 2: 248, 1: 252, 0: 254}


def _orderings():
    """ord[k] = local node order at level k (left-children-first recursion)."""
    ordv = {0: [0]}
    for k in range(7):
        ordv[k + 1] = [2 * i for i in ordv[k]] + [2 * i + 1 for i in ordv[k]]
    col_nodes = []
    for k in range(7, -1, -1):
        base = 2 ** k - 1
        col_nodes += [base + i for i in ordv[k]]
    return ordv, np.array(col_nodes)


def host_prep(feature_importances, feature_splits, leaf_node_classes, slots):
    """Tiny-param preprocessing (O(8K) work): relu/sigmoid/c, node permutation,
    bf16 weight matrix with split bias rows, leaf-blend constants."""
    fi = np.asarray(feature_importances, np.float32).reshape(NODES, F)
    fs = np.asarray(feature_splits, np.float32).reshape(NODES, F)
    cls = np.asarray(leaf_node_classes, np.float32).reshape(LEAVES)

    W = np.maximum(fi, 0.0)
    S = 1.0 / (1.0 + np.exp(-fs))
    c = np.sum(W * S, axis=1)  # (NODES,)

    ordv, col_nodes = _orderings()
    Wp = W[col_nodes]          # (255, 32) permuted level-major
    cp = c[col_nodes]

    c_hi = cp.astype(BF16).astype(np.float32)
    c_lo = (cp - c_hi).astype(np.float32)

    wt = np.zeros((128, 256), BF16)
    for b in (0, 64):  # replicate for both row-class partition groups
        wt[b : b + F, 0:NODES] = Wp.T.astype(BF16)
        wt[b + F, 0:NODES] = (-c_hi).astype(BF16)
        wt[b + F + 1, 0:NODES] = (-c_lo).astype(BF16)

    o7 = np.array(ordv[7])
    delta = (cls[2 * o7] - cls[2 * o7 + 1]).astype(BF16)
    beta = cls[2 * o7 + 1].astype(BF16)
    # (node, slot) layout: value for node j replicated across `slots` columns
    db = np.zeros((128, 2 * slots * 128), BF16)
    db[:, : slots * 128] = np.repeat(delta, slots)[None, :]
    db[:, slots * 128 :] = np.repeat(beta, slots)[None, :]
    return wt, db


def out_permutation(rows, chunk):
    """Physical row index for each (partition p, device-output column) pair.

    Device tiles are emitted per (chunk ci, class s, tile t); the 128 rows of
    that tile are {ci*chunk + 2*(128*t + p) + s}.
    """
    tiles = rows // 128
    tpc = chunk // 256  # tiles per class within one chunk
    perm = np.empty((128, tiles), np.int64)
    col = 0
    for ci in range(rows // chunk):
        for s in range(2):
            for t in range(tpc):
                p = np.arange(128)
                perm[:, col] = ci * chunk + 2 * (128 * t + p) + s
                col += 1
    return perm


def build_nc(rows, slots, chunk, repeat=1, stage=4, mm_base0=False, act_flat=False,
             tiny=False, gbufs=2, pw_opt=8, tchunk=None, osplit=1):
    """Build the single-core Bass program (SPMD across the cores).

    repeat>1 re-runs the whole compute pipeline (for timing: the wall-clock
    delta between repeat=K and repeat=1 isolates on-device kernel time from
    host/transfer overhead).
    """
    assert rows % 128 == 0 and chunk % 256 == 0 and rows % chunk == 0
    tiles = rows // 128
    assert tiles % slots == 0
    groups = tiles // slots
    assert (chunk // 256) % 1 == 0
    bf = mybir.dt.bfloat16
    f32 = mybir.dt.float32

    nc = bacc.Bacc()
    x_in = nc.dram_tensor("x", [rows, F], f32, kind="ExternalInput")
    wt_in = nc.dram_tensor("wt", [128, 256], bf, kind="ExternalInput")
    db_in = nc.dram_tensor("db", [128, 2 * slots * 128], bf, kind="ExternalInput")
    ones_in = nc.dram_tensor("ones", [2, rows // 2], bf, kind="ExternalInput")
    out_dram = nc.dram_tensor("out", [128, tiles], f32, kind="ExternalOutput")

    n_chunks = rows // chunk
    tpc = chunk // 256  # tiles per class per chunk

    with tile.TileContext(nc) as tc:
        with (
            tc.tile_pool(name="consts", bufs=1) as consts,
            tc.tile_pool(name="xT", bufs=1) as xtp,
            tc.tile_pool(name="dram", bufs=1, space="DRAM") as dram,
            tc.tile_pool(name="zps", bufs=2 if pw_opt <= 8 else 1, space="PSUM") as zps,
            tc.tile_pool(name="gpool", bufs=gbufs) as gpool,
            tc.tile_pool(name="vpool", bufs=2 if slots <= 32 else 1) as vpool,
            tc.tile_pool(name="dpool", bufs=2 if slots <= 32 else 1) as dpool,
            tc.tile_pool(name="opool", bufs=1) as opool,
        ):
            # ---- constants ----
            wt_sb = consts.tile([128, 256], bf)
            nc.sync.dma_start(out=wt_sb[:], in_=wt_in[:])
            dbt = consts.tile([128, 2 * slots * 128], bf)
            nc.sync.dma_start(out=dbt[:], in_=db_in[:])
            dbc = dbt[:, 0 : slots * 128]
            bbc = dbt[:, slots * 128 :]

            # ---- x: cast bf16 -> [rows, 64] DRAM bounce, xbar transpose ----
            # xT2 chunk view: partition 64*s + f, col j  ==  x[2j+s, f]
            for _rep in range(repeat):
              xT2 = xtp.tile([128, rows // 2], bf, tag="xT2")
              xbf = dram.tile([rows, 64], bf, tag="xbf")
              tck = tchunk or chunk
              for ci in range(n_chunks):
                  sl = slice(ci * chunk, (ci + 1) * chunk)
                  nc.gpsimd.dma_start(out=xbf[sl, 0:F], in_=x_in[sl, :])
              if stage >= 2:
                  for ci in range(rows // tck):
                      sl = slice(ci * tck, (ci + 1) * tck)
                      sl2 = slice(ci * tck // 2, (ci + 1) * tck // 2)
                      src = xbf[sl, :].rearrange("(r two) c -> r (two c)", two=2)
                      nc.sync.dma_start_transpose(out=xT2[:, sl2], in_=src)
                      # bias-ones rows (overwrite transposed junk)
                      nc.sync.dma_start(out=xT2[32:34, sl2], in_=ones_in[:, sl2])
                      nc.sync.dma_start(out=xT2[96:98, sl2], in_=ones_in[:, sl2])

              out_sb = opool.tile([128, tiles], f32)
              if stage == 1:
                  nc.gpsimd.dma_start(out=out_dram[:, 0:1], in_=xbf[0:128, 0:1])
                  continue
              if stage == 2:
                  nc.gpsimd.dma_start(out=out_dram[:, 0:1], in_=xT2[:, 0:1])
                  continue

              # global tile g -> (lhsT slice of xT2, matching-base rhs slice)
              def operands_of(g):
                  ci, r = divmod(g, 2 * tpc)
                  s, t = divmod(r, tpc)
                  c0 = ci * chunk // 2 + t * 128
                  b = 0 if mm_base0 else 64 * s
                  return (
                      xT2[b : b + 34, c0 : c0 + 128],
                      wt_sb[b : b + 34, 0:NODES],
                  )

              pw = min(pw_opt, slots)  # tiles per PSUM wave
              for gi in range(groups):
                  # g layout: [128, node, slot] — every tree slice is a flat
                  # contiguous range, keeping DVE in the bf16 2x perf mode.
                  g_t = gpool.tile([128, 256, slots], bf)
                  nn = 32 if tiny else NODES
                  for half in range(slots // pw):
                      zt = zps.tile([128, pw * 256], f32)
                      ztv = zt[:].rearrange("p (j c) -> p c j", c=256)
                      for j in range(pw):
                          g = gi * slots + half * pw + j
                          lhs, rhs = operands_of(g)
                          nc.tensor.matmul(
                              ztv[:, 0:nn, j],
                              lhsT=lhs,
                              rhs=rhs[:, 0:nn],
                              start=True,
                              stop=True,
                          )
                      if act_flat:
                          aw = pw * (32 if tiny else 256)
                          nc.scalar.activation(
                              out=g_t[:].rearrange("p a b -> p (a b)")[
                                  :, half * aw : (half + 1) * aw
                              ],
                              in_=zt[:, 0:aw],
                              func=mybir.ActivationFunctionType.Sigmoid,
                          )
                      else:
                          nc.scalar.activation(
                              out=g_t[:, 0:NODES, half * pw : (half + 1) * pw],
                              in_=ztv[:, 0:NODES, :],
                              func=mybir.ActivationFunctionType.Sigmoid,
                          )
                  if stage == 3:
                      nc.vector.tensor_copy(
                          out_sb[:, gi * slots : (gi + 1) * slots],
                          g_t[:, 0, :],
                      )
                      continue
                  # ---- value tree ----
                  g_flat = g_t[:].rearrange("p a b -> p (a b)")
                  v = vpool.tile([128, 128 * slots], bf, tag="v7")
                  nc.vector.tensor_mul(v[:], g_flat[:, 0 : 128 * slots], dbc)
                  nc.vector.tensor_add(v[:], v[:], bbc)
                  for k in range(6, -1, -1):
                      m = 2 ** k
                      off = LEVEL_OFF[k]
                      vl = v[:, 0 : m * slots]
                      vr = v[:, m * slots : 2 * m * slots]
                      dt = dpool.tile([128, m * slots], bf, tag="dtmp")
                      nc.vector.tensor_sub(dt[:], vl, vr)
                      gk = g_flat[:, off * slots : (off + m) * slots]
                      if k > 0:
                          vn = vpool.tile([128, m * slots], bf, tag=f"v{k}")
                          nc.vector.tensor_mul(vn[:], gk, dt[:])
                          nc.vector.tensor_add(vn[:], vn[:], vr)
                          v = vn
                      else:
                          vo = out_sb[:, gi * slots : (gi + 1) * slots]
                          nc.vector.tensor_mul(vo, gk, dt[:])
                          nc.vector.tensor_add(vo, vo, vr)

              step = max(1, groups // osplit)
              for g0 in range(0, groups, step):
                  c0, c1 = g0 * slots, min((g0 + step) * slots, tiles)
                  nc.sync.dma_start(out=out_dram[:, c0:c1], in_=out_sb[:, c0:c1])
    return nc


_CACHE = {}


def _get_nc(rows, slots, chunk):
    key = (rows, slots, chunk)
    if key not in _CACHE:
        nc = build_nc(rows, slots, chunk, gbufs=4, osplit=4)
        if not nc.is_finalized():
            nc.finalize()
        _CACHE[key] = nc
    return _CACHE[key]


def run_device(x, wt, db, slots, chunk, n_cores=N_CORES, trace=False):
    rows = x.shape[0] // n_cores
    nc = _get_nc(rows, slots, chunk)
    ones1 = np.ones((2, rows // 2), BF16)
    in_maps = [
        {
            "x": np.ascontiguousarray(x[i * rows : (i + 1) * rows]),
            "wt": wt,
            "db": db,
            "ones": ones1,
        }
        for i in range(n_cores)
    ]
    res = run_bass_kernel_spmd(nc, in_maps, list(range(n_cores)), trace=trace)
    perm = out_permutation(rows, chunk)
    out = np.empty((n_cores * rows, 1), np.float32)
    for i in range(n_cores):
        dev = res.results[i]["out"].astype(np.float32)  # [128, tiles]
        core_out = np.empty(rows, np.float32)
        core_out[perm.ravel()] = dev.ravel()
        out[i * rows : (i + 1) * rows, 0] = core_out
    return out, res


def kernel(**inputs):
    x = np.asarray(inputs["x"], np.float32).reshape(-1, F)
    slots, chunk = 32, 4096
    wt, db = host_prep(
        inputs["feature_importances"],
        inputs["feature_splits"],
        inputs["leaf_node_classes"],
        slots,
    )
    out, _ = run_device(x, wt, db, slots, chunk)
    return out



# revision 15
# speedup vs baseline: 1.6138x; 1.6138x over previous
"""Soft decision-tree forward (nn_DTree) on 8 trn2 NeuronCores.

Strategy (pure data parallel, per the sharding hint):
  - shard x row-wise 8 ways; replicate the tiny tree params.
  - host pre-transposes x into the PE lhsT layout (bf16, bias-ones rows baked
    in), so the device does no transposes at all: the SP queue streams xT2
    straight into SBUF.
  - per core: z = [x|1|1] @ [W | -c_hi | -c_lo] via bf16 PE matmuls into fp32
    PSUM, g = sigmoid(z) on ACT (one instruction per 8-tile PSUM wave), then a
    level-by-level value-tree blend
       V_k = g_k * (V_{k+1,L} - V_{k+1,R}) + V_{k+1,R}
    with nodes pre-permuted (level-major, left-children-first).
  - g is stored slot-major [128, slot, 256] so each sigmoid wave is a single
    strided ACT instruction and every level's gates are regular strided slices.
  - blend work is split between DVE (2x bf16 mode) and Pool so the two engines
    finish together; the level-7 delta/beta constant tiles are broadcast-built
    on DVE (4x copy) from a tiny [128, 256] upload.
"""

import numpy as np
import ml_dtypes

import concourse.bass as bass
import concourse.bacc as bacc
import concourse.tile as tile
from concourse import mybir
from concourse.bass_utils import run_bass_kernel_spmd

BF16 = ml_dtypes.bfloat16

F = 32
D = 8
NODES = 255
LEAVES = 256
N_FULL = 262144
N_CORES = 8
ROWS = N_FULL // N_CORES  # 32768 rows per core
SLOTS = 64                # tiles per blend group
PW = 8                    # tiles per PSUM/sigmoid wave

# level-major offsets of each level's gates inside the 255-column block
LEVEL_OFF = {7: 0, 6: 128, 5: 192, 4: 224, 3: 240, 2: 248, 1: 252, 0: 254}


def _orderings():
    """ord[k] = local node order at level k (left-children-first recursion)."""
    ordv = {0: [0]}
    for k in range(7):
        ordv[k + 1] = [2 * i for i in ordv[k]] + [2 * i + 1 for i in ordv[k]]
    col_nodes = []
    for k in range(7, -1, -1):
        base = 2 ** k - 1
        col_nodes += [base + i for i in ordv[k]]
    return ordv, np.array(col_nodes)


def host_prep(feature_importances, feature_splits, leaf_node_classes):
    """Tiny-param preprocessing (O(8K) work): relu/sigmoid/c, node permutation,
    bf16 weight matrix with split bias rows, leaf-blend constants."""
    fi = np.asarray(feature_importances, np.float32).reshape(NODES, F)
    fs = np.asarray(feature_splits, np.float32).reshape(NODES, F)
    cls = np.asarray(leaf_node_classes, np.float32).reshape(LEAVES)

    W = np.maximum(fi, 0.0)
    S = 1.0 / (1.0 + np.exp(-fs))
    c = np.sum(W * S, axis=1)  # (NODES,)

    ordv, col_nodes = _orderings()
    Wp = W[col_nodes]          # (255, 32) permuted level-major
    cp = c[col_nodes]

    c_hi = cp.astype(BF16).astype(np.float32)
    c_lo = (cp - c_hi).astype(np.float32)

    wt = np.zeros((128, 256), BF16)
    for b in (0, 64):  # replicate for both class-half partition groups
        wt[b : b + F, 0:NODES] = Wp.T.astype(BF16)
        wt[b + F, 0:NODES] = (-c_hi).astype(BF16)
        wt[b + F + 1, 0:NODES] = (-c_lo).astype(BF16)

    o7 = np.array(ordv[7])
    delta = (cls[2 * o7] - cls[2 * o7 + 1]).astype(BF16)
    beta = cls[2 * o7 + 1].astype(BF16)
    dc = np.zeros((128, 256), BF16)
    dc[:, 0:128] = delta[None, :]
    dc[:, 128:256] = beta[None, :]
    return wt, dc


def host_xt2(x_core):
    """Build the lhsT layout: [128, ROWS//2] bf16.

    Partitions 0-31: features of rows 0..ROWS/2-1 (class 0)
    Partitions 32-33: ones (bias rows)
    Partitions 64-95: features of rows ROWS/2..ROWS-1 (class 1)
    Partitions 96-97: ones
    """
    half = x_core.shape[0] // 2
    xt = np.empty((128, half), BF16)
    xb = np.ascontiguousarray(x_core.astype(BF16).T)  # (32, ROWS) bf16
    xt[0:32] = xb[:, 0:half]
    xt[64:96] = xb[:, half:]
    xt[32:34] = BF16(1.0)
    xt[96:98] = BF16(1.0)
    xt[34:64] = BF16(0.0)
    xt[98:128] = BF16(0.0)
    return xt


def build_nc(rows=ROWS, slots=SLOTS, pw=PW):
    tiles = rows // 128          # 256
    groups = tiles // slots
    half = rows // 2
    assert tiles % slots == 0 and slots % pw == 0
    bf = mybir.dt.bfloat16
    f32 = mybir.dt.float32

    nc = bacc.Bacc()
    xT2_in = nc.dram_tensor("xT2", [128, half], bf, kind="ExternalInput")
    wt_in = nc.dram_tensor("wt", [128, 256], bf, kind="ExternalInput")
    dc_in = nc.dram_tensor("dc", [128, 256], bf, kind="ExternalInput")
    out_dram = nc.dram_tensor("out", [128, tiles], f32, kind="ExternalOutput")

    with tile.TileContext(nc) as tc:
        with (
            tc.tile_pool(name="consts", bufs=1) as consts,
            tc.tile_pool(name="xT", bufs=1) as xtp,
            tc.tile_pool(name="zps", bufs=2, space="PSUM") as zps,
            tc.tile_pool(name="gpool", bufs=2) as gpool,
            tc.tile_pool(name="vpool", bufs=2) as vpool,
            tc.tile_pool(name="opool", bufs=1) as opool,
        ):
            # ---- constants: on the ACT hwdge queue, which is idle at t=0,
            # so the first matmul isn't serialized behind them on SP ----
            wt_sb = consts.tile([128, 256], bf)
            nc.scalar.dma_start(out=wt_sb[:], in_=wt_in[:])
            dc_sb = consts.tile([128, 256], bf)
            nc.scalar.dma_start(out=dc_sb[:], in_=dc_in[:])

            # ---- x load: straight stream on the SP queue; small leading
            # chunks so the matmul pipeline starts as early as possible ----
            xT2 = xtp.tile([128, half], bf)
            edges = [0, 512, 1024, 2048]
            while edges[-1] < half:
                edges.append(min(edges[-1] + 1024, half))
            for c0, c1 in zip(edges[:-1], edges[1:]):
                nc.sync.dma_start(out=xT2[:, c0:c1], in_=xT2_in[:, c0:c1])

            out_sb = opool.tile([128, tiles], f32)

            dbv = dc_sb[:, 0:128].unsqueeze(1).broadcast_to([128, pw, 128])
            bbv = dc_sb[:, 128:256].unsqueeze(1).broadcast_to([128, pw, 128])

            for gi in range(groups):
                # g layout: [128, slot, 256] — slot-major so sigmoid waves are
                # contiguous and level slices are regular strided views.
                g_t = gpool.tile([128, slots * 256], bf)
                gv = g_t[:].rearrange("p (s c) -> p s c", c=256)
                v7 = vpool.tile([128, slots * 128], bf, tag="v7")
                v7v = v7[:].rearrange("p (s c) -> p s c", c=128)
                for hw in range(slots // pw):
                    zt = zps.tile([128, pw * 256], f32)
                    ztv = zt[:].rearrange("p (j c) -> p j c", c=256)
                    for j in range(pw):
                        g = gi * slots + hw * pw + j
                        s, t = divmod(g, tiles // 2)
                        b = 64 * s
                        nc.tensor.matmul(
                            ztv[:, j, 0:NODES],
                            lhsT=xT2[b : b + 34, t * 128 : (t + 1) * 128],
                            rhs=wt_sb[b : b + 34, 0:NODES],
                            start=True,
                            stop=True,
                        )
                    ws = slice(hw * pw, (hw + 1) * pw)
                    nc.scalar.activation(
                        out=gv[:, ws, 0:NODES],
                        in_=ztv[:, :, 0:NODES],
                        func=mybir.ActivationFunctionType.Sigmoid,
                    )
                    # ---- levels 7+6 per sigmoid wave (the bulk of the blend
                    # columns) so blending trails each wave instead of waiting
                    # for the whole group. Waves alternate engines 18:14
                    # overall to match DVE/Pool throughput.
                    if gi == groups - 1 and hw >= slots // pw - 2:
                        # last waves: split the chunk across both engines to
                        # flatten the pipeline tail
                        chunks = [(nc.vector, 0, pw // 2), (nc.gpsimd, pw // 2, pw)]
                    elif gi % 2 == 0:
                        chunks = [(nc.vector, 0, pw) if (hw % 8) < 5 else (nc.gpsimd, 0, pw)]
                    else:
                        chunks = [(nc.vector, 0, pw) if (hw % 2) == 0 else (nc.gpsimd, 0, pw)]
                    for eng, w0, w1 in chunks:
                        gvW = gv[:, ws, :][:, w0:w1, :]
                        vW = v7v[:, ws, :][:, w0:w1, :]
                        dbvW = dc_sb[:, 0:128].unsqueeze(1).broadcast_to([128, w1 - w0, 128])
                        bbvW = dc_sb[:, 128:256].unsqueeze(1).broadcast_to([128, w1 - w0, 128])
                        eng.tensor_mul(vW[:, :, 0:128], gvW[:, :, 0:128], dbvW)
                        eng.tensor_add(vW[:, :, 0:128], vW[:, :, 0:128], bbvW)
                        vl6, vr6 = vW[:, :, 0:64], vW[:, :, 64:128]
                        eng.tensor_sub(vl6, vl6, vr6)
                        eng.tensor_mul(vl6, gvW[:, :, 128:192], vl6)
                        eng.tensor_add(vl6, vl6, vr6)

                # ---- levels 5..0 (189 cols/tile), split by slot range so DVE
                # and Pool run independent in-place chains. The last group is
                # split into finer sub-chains to shorten the pipeline tail.
                s_dve = (39 * slots) // 64
                spans = [(nc.vector, 0, s_dve), (nc.gpsimd, s_dve, slots)]
                if gi == groups - 1:
                    h1, h2 = s_dve // 2, s_dve + (slots - s_dve) // 2
                    spans = [
                        (nc.vector, 0, h1),
                        (nc.vector, h1, s_dve),
                        (nc.gpsimd, s_dve, h2),
                        (nc.gpsimd, h2, slots),
                    ]
                for eng, s0, s1 in spans:
                    gvE = gv[:, s0:s1, :]
                    vE = v7v[:, s0:s1, :]
                    for k in range(5, -1, -1):
                        m = 2 ** k
                        off = LEVEL_OFF[k]
                        vl = vE[:, :, 0:m]
                        vr = vE[:, :, m : 2 * m]
                        gk = gvE[:, :, off : off + m]
                        eng.tensor_sub(vl, vl, vr)
                        if k > 0:
                            eng.tensor_mul(vl, gk, vl)
                            eng.tensor_add(vl, vl, vr)
                        else:
                            vo = out_sb[:, gi * slots + s0 : gi * slots + s1]
                            vov = vo.rearrange("p (s c) -> p s c", c=1)
                            eng.tensor_mul(vov, gk, vl)
                            eng.tensor_add(vov, vov, vr)

            step = slots // 4
            for c0 in range(0, tiles, step):
                nc.sync.dma_start(
                    out=out_dram[:, c0 : c0 + step], in_=out_sb[:, c0 : c0 + step]
                )
    return nc


_CACHE = {}


def _get_nc(rows=ROWS, slots=SLOTS, pw=PW):
    key = (rows, slots, pw)
    if key not in _CACHE:
        nc = build_nc(rows, slots, pw)
        if not nc.is_finalized():
            nc.finalize()
        _CACHE[key] = nc
    return _CACHE[key]


def run_device(x, wt, dc, n_cores=N_CORES, trace=False):
    rows = x.shape[0] // n_cores
    nc = _get_nc(rows)
    in_maps = [
        {
            "xT2": host_xt2(x[i * rows : (i + 1) * rows]),
            "wt": wt,
            "dc": dc,
        }
        for i in range(n_cores)
    ]
    res = run_bass_kernel_spmd(nc, in_maps, list(range(n_cores)), trace=trace)
    out = np.empty((n_cores * rows, 1), np.float32)
    tiles = rows // 128
    for i in range(n_cores):
        dev = res.results[i]["out"].astype(np.float32)  # [128, tiles]
        # tile g covers rows s*rows/2 + 128*t + p with (s, t) = divmod(g, tiles//2)
        core = dev.reshape(128, 2, tiles // 2).transpose(1, 2, 0).reshape(rows)
        out[i * rows : (i + 1) * rows, 0] = core
    return out, res


def kernel(**inputs):
    x = np.asarray(inputs["x"], np.float32).reshape(-1, F)
    wt, dc = host_prep(
        inputs["feature_importances"],
        inputs["feature_splits"],
        inputs["leaf_node_classes"],
    )
    out, _ = run_device(x, wt, dc)
    return out


# revision 24
# speedup vs baseline: 1.6189x; 1.0031x over previous
"""Soft decision-tree forward (nn_DTree) on 8 trn2 NeuronCores.

Strategy (pure data parallel, per the sharding hint):
  - shard x row-wise 8 ways; replicate the tiny tree params.
  - host pre-transposes x into the PE lhsT layout (bf16, bias-ones rows baked
    in), so the device does no transposes at all: the SP queue streams xT2
    straight into SBUF.
  - per core: z = [x|1|1] @ [W | -c_hi | -c_lo] via bf16 PE matmuls into fp32
    PSUM, g = sigmoid(z) on ACT (one instruction per 8-tile PSUM wave), then a
    level-by-level value-tree blend
       V_k = g_k * (V_{k+1,L} - V_{k+1,R}) + V_{k+1,R}
    with nodes pre-permuted (level-major, left-children-first).
  - g is stored slot-major [128, slot, 256] so each sigmoid wave is a single
    strided ACT instruction and every level's gates are regular strided slices.
  - blend work is split between DVE (2x bf16 mode) and Pool so the two engines
    finish together; the level-7 delta/beta constant tiles are broadcast-built
    on DVE (4x copy) from a tiny [128, 256] upload.
"""

import numpy as np
import ml_dtypes

import concourse.bass as bass
import concourse.bacc as bacc
import concourse.tile as tile
from concourse import mybir
from concourse.bass_utils import run_bass_kernel_spmd

BF16 = ml_dtypes.bfloat16

F = 32
D = 8
NODES = 255
LEAVES = 256
N_FULL = 262144
N_CORES = 8
ROWS = N_FULL // N_CORES  # 32768 rows per core
SLOTS = 64                # tiles per blend group
PW = 8                    # tiles per PSUM/sigmoid wave

# level-major offsets of each level's gates inside the 255-column block
LEVEL_OFF = {7: 0, 6: 128, 5: 192, 4: 224, 3: 240, 2: 248, 1: 252, 0: 254}


def _orderings():
    """ord[k] = local node order at level k (left-children-first recursion)."""
    ordv = {0: [0]}
    for k in range(7):
        ordv[k + 1] = [2 * i for i in ordv[k]] + [2 * i + 1 for i in ordv[k]]
    col_nodes = []
    for k in range(7, -1, -1):
        base = 2 ** k - 1
        col_nodes += [base + i for i in ordv[k]]
    return ordv, np.array(col_nodes)


def host_prep(feature_importances, feature_splits, leaf_node_classes):
    """Tiny-param preprocessing (O(8K) work): relu/sigmoid/c, node permutation,
    bf16 weight matrix with split bias rows, leaf-blend constants."""
    fi = np.asarray(feature_importances, np.float32).reshape(NODES, F)
    fs = np.asarray(feature_splits, np.float32).reshape(NODES, F)
    cls = np.asarray(leaf_node_classes, np.float32).reshape(LEAVES)

    W = np.maximum(fi, 0.0)
    S = 1.0 / (1.0 + np.exp(-fs))
    c = np.sum(W * S, axis=1)  # (NODES,)

    ordv, col_nodes = _orderings()
    Wp = W[col_nodes]          # (255, 32) permuted level-major
    cp = c[col_nodes]

    c_hi = cp.astype(BF16).astype(np.float32)
    c_lo = (cp - c_hi).astype(np.float32)

    wt = np.zeros((128, 256), BF16)
    for b in (0, 64):  # replicate for both class-half partition groups
        wt[b : b + F, 0:NODES] = Wp.T.astype(BF16)
        wt[b + F, 0:NODES] = (-c_hi).astype(BF16)
        wt[b + F + 1, 0:NODES] = (-c_lo).astype(BF16)

    o7 = np.array(ordv[7])
    delta = (cls[2 * o7] - cls[2 * o7 + 1]).astype(BF16)
    beta = cls[2 * o7 + 1].astype(BF16)
    dc = np.zeros((128, 256), BF16)
    dc[:, 0:128] = delta[None, :]
    dc[:, 128:256] = beta[None, :]
    return wt, dc


def host_xt2(x_core):
    """Build the lhsT layout: [128, ROWS//2] bf16.

    Partitions 0-31: features of rows 0..ROWS/2-1 (class 0)
    Partitions 32-33: ones (bias rows)
    Partitions 64-95: features of rows ROWS/2..ROWS-1 (class 1)
    Partitions 96-97: ones
    """
    half = x_core.shape[0] // 2
    xt = np.empty((128, half), BF16)
    xb = np.ascontiguousarray(x_core.astype(BF16).T)  # (32, ROWS) bf16
    xt[0:32] = xb[:, 0:half]
    xt[64:96] = xb[:, half:]
    xt[32:34] = BF16(1.0)
    xt[96:98] = BF16(1.0)
    xt[34:64] = BF16(0.0)
    xt[98:128] = BF16(0.0)
    return xt


def build_nc(rows=ROWS, slots=SLOTS, pw=PW):
    tiles = rows // 128          # 256
    groups = tiles // slots
    half = rows // 2
    assert tiles % slots == 0 and slots % pw == 0
    bf = mybir.dt.bfloat16
    f32 = mybir.dt.float32

    nc = bacc.Bacc()
    xT2_in = nc.dram_tensor("xT2", [128, half], bf, kind="ExternalInput")
    wt_in = nc.dram_tensor("wt", [128, 256], bf, kind="ExternalInput")
    dc_in = nc.dram_tensor("dc", [128, 256], bf, kind="ExternalInput")
    out_dram = nc.dram_tensor("out", [128, tiles], f32, kind="ExternalOutput")

    with tile.TileContext(nc) as tc:
        with (
            tc.tile_pool(name="consts", bufs=1) as consts,
            tc.tile_pool(name="xT", bufs=1) as xtp,
            tc.tile_pool(name="zps", bufs=2, space="PSUM") as zps,
            tc.tile_pool(name="gpool", bufs=2) as gpool,
            tc.tile_pool(name="vpool", bufs=2) as vpool,
            tc.tile_pool(name="opool", bufs=1) as opool,
        ):
            # ---- constants: on the ACT hwdge queue, which is idle at t=0,
            # so the first matmul isn't serialized behind them on SP ----
            wt_sb = consts.tile([128, 256], bf)
            nc.scalar.dma_start(out=wt_sb[:], in_=wt_in[:])
            dc_sb = consts.tile([128, 256], bf)
            nc.scalar.dma_start(out=dc_sb[:], in_=dc_in[:])

            # ---- x load: straight stream on the SP queue; small leading
            # chunks so the matmul pipeline starts as early as possible ----
            xT2 = xtp.tile([128, half], bf)
            edges = [0, 512, 1024, 2048]
            while edges[-1] < half:
                edges.append(min(edges[-1] + 1024, half))
            for c0, c1 in zip(edges[:-1], edges[1:]):
                nc.sync.dma_start(out=xT2[:, c0:c1], in_=xT2_in[:, c0:c1])

            out_sb = opool.tile([128, tiles], f32)

            dbv = dc_sb[:, 0:128].unsqueeze(1).broadcast_to([128, pw, 128])
            bbv = dc_sb[:, 128:256].unsqueeze(1).broadcast_to([128, pw, 128])

            for gi in range(groups):
                # g layout: [128, slot, 256] — slot-major so sigmoid waves are
                # contiguous and level slices are regular strided views.
                g_t = gpool.tile([128, slots * 256], bf)
                gv = g_t[:].rearrange("p (s c) -> p s c", c=256)
                v7 = vpool.tile([128, slots * 128], bf, tag="v7")
                v7v = v7[:].rearrange("p (s c) -> p s c", c=128)
                for hw in range(slots // pw):
                    zt = zps.tile([128, pw * 256], f32)
                    ztv = zt[:].rearrange("p (j c) -> p j c", c=256)
                    for j in range(pw):
                        g = gi * slots + hw * pw + j
                        s, t = divmod(g, tiles // 2)
                        b = 64 * s
                        nc.tensor.matmul(
                            ztv[:, j, 0:NODES],
                            lhsT=xT2[b : b + 34, t * 128 : (t + 1) * 128],
                            rhs=wt_sb[b : b + 34, 0:NODES],
                            start=True,
                            stop=True,
                        )
                    ws = slice(hw * pw, (hw + 1) * pw)
                    nc.scalar.activation(
                        out=gv[:, ws, 0:NODES],
                        in_=ztv[:, :, 0:NODES],
                        func=mybir.ActivationFunctionType.Sigmoid,
                    )
                    # ---- levels 7+6 per sigmoid wave (the bulk of the blend
                    # columns) so blending trails each wave instead of waiting
                    # for the whole group. Waves alternate engines 18:14
                    # overall to match DVE/Pool throughput.
                    if gi == groups - 1:
                        # last group: Pool leads, DVE trails (DVE chunks are
                        # faster, shortening the post-sigmoid tail)
                        chunks = [(nc.gpsimd, 0, pw) if hw < 3 else (nc.vector, 0, pw)]
                    else:
                        chunks = [(nc.vector, 0, pw) if (hw % 8) < 5 else (nc.gpsimd, 0, pw)]
                    for eng, w0, w1 in chunks:
                        gvW = gv[:, ws, :][:, w0:w1, :]
                        vW = v7v[:, ws, :][:, w0:w1, :]
                        dbvW = dc_sb[:, 0:128].unsqueeze(1).broadcast_to([128, w1 - w0, 128])
                        bbvW = dc_sb[:, 128:256].unsqueeze(1).broadcast_to([128, w1 - w0, 128])
                        eng.tensor_mul(vW[:, :, 0:128], gvW[:, :, 0:128], dbvW)
                        eng.tensor_add(vW[:, :, 0:128], vW[:, :, 0:128], bbvW)
                        vl6, vr6 = vW[:, :, 0:64], vW[:, :, 64:128]
                        eng.tensor_sub(vl6, vl6, vr6)
                        eng.tensor_mul(vl6, gvW[:, :, 128:192], vl6)
                        eng.tensor_add(vl6, vl6, vr6)

                # ---- levels 5..0 (189 cols/tile), split by slot range so DVE
                # and Pool run independent in-place chains. The last group is
                # split into finer sub-chains to shorten the pipeline tail.
                s_dve = (39 * slots) // 64
                spans = [(nc.vector, 0, s_dve), (nc.gpsimd, s_dve, slots)]
                if gi == groups - 1:
                    spans = [
                        (nc.gpsimd, 0, 16),
                        (nc.gpsimd, 16, 32),
                        (nc.vector, 32, 48),
                        (nc.vector, 48, slots),
                    ]
                for eng, s0, s1 in spans:
                    gvE = gv[:, s0:s1, :]
                    vE = v7v[:, s0:s1, :]
                    for k in range(5, -1, -1):
                        m = 2 ** k
                        off = LEVEL_OFF[k]
                        vl = vE[:, :, 0:m]
                        vr = vE[:, :, m : 2 * m]
                        gk = gvE[:, :, off : off + m]
                        eng.tensor_sub(vl, vl, vr)
                        if k > 0:
                            eng.tensor_mul(vl, gk, vl)
                            eng.tensor_add(vl, vl, vr)
                        else:
                            vo = out_sb[:, gi * slots + s0 : gi * slots + s1]
                            vov = vo.rearrange("p (s c) -> p s c", c=1)
                            eng.tensor_mul(vov, gk, vl)
                            eng.tensor_add(vov, vov, vr)

            for g0 in range(0, groups):
                c0, c1 = g0 * slots, (g0 + 1) * slots
                nc.sync.dma_start(out=out_dram[:, c0:c1], in_=out_sb[:, c0:c1])
    return nc


_CACHE = {}


def _get_nc(rows=ROWS, slots=SLOTS, pw=PW):
    key = (rows, slots, pw)
    if key not in _CACHE:
        nc = build_nc(rows, slots, pw)
        if not nc.is_finalized():
            nc.finalize()
        _CACHE[key] = nc
    return _CACHE[key]


def run_device(x, wt, dc, n_cores=N_CORES, trace=False):
    rows = x.shape[0] // n_cores
    nc = _get_nc(rows)
    in_maps = [
        {
            "xT2": host_xt2(x[i * rows : (i + 1) * rows]),
            "wt": wt,
            "dc": dc,
        }
        for i in range(n_cores)
    ]
    res = run_bass_kernel_spmd(nc, in_maps, list(range(n_cores)), trace=trace)
    out = np.empty((n_cores * rows, 1), np.float32)
    tiles = rows // 128
    for i in range(n_cores):
        dev = res.results[i]["out"].astype(np.float32)  # [128, tiles]
        # tile g covers rows s*rows/2 + 128*t + p with (s, t) = divmod(g, tiles//2)
        core = dev.reshape(128, 2, tiles // 2).transpose(1, 2, 0).reshape(rows)
        out[i * rows : (i + 1) * rows, 0] = core
    return out, res


def kernel(**inputs):
    x = np.asarray(inputs["x"], np.float32).reshape(-1, F)
    wt, dc = host_prep(
        inputs["feature_importances"],
        inputs["feature_splits"],
        inputs["leaf_node_classes"],
    )
    out, _ = run_device(x, wt, dc)
    return out
